# revision 13
# baseline (speedup 1.0000x reference)
"""Multi-head graph-attention layer for Trainium2 (8-core SPMD).

The reference computes per-head projections hp = einsum("bnf,hfd->bhnd", h, W),
dense attention scores e = hp @ hp^T, LeakyReLU, softmax over the last axis,
and then multiplies hp by sum_j(softmax(e))_j. The sum of a softmax over its
own normalization axis is identically 1, so the layer's exact mathematical
output is hp itself (concatenated over heads):

    out[b, n, h*64+d] = sum_f h[b,n,f] * W[h,f,d]  =  (h[b] @ Wc)[n, h*64+d]

with Wc[f, h*64+d] = W[h,f,d]. The reference's deviation from rowsum==1 is
fp32 rounding noise (~1e-6 relative) that no reimplementation reproduces, so
computing the projection directly is both the fastest and the most accurate
realization. `adj` is unused by the reference and is ignored here.

Sharding: data-parallel over the batch dim B=8, one graph per NeuronCore.
Each core computes Y[b]^T = (Wc^T @ h[b]^T) as a [256,256] x [256,2048]
matmul with Wc chunks stationary on the PE (float32r: single-pass reduced-
precision fp32, 1 cycle/row). Inputs are host-transposed to [F_IN, N] so
every DMA is fully contiguous. Warm-up matmuls run on scratch data during
the input-DMA wait so the real matmuls hit the 2.4 GHz warm clock.
"""

import numpy as np

import concourse.bass as bass
import concourse.mybir as mybir
import concourse.tile as tile
from concourse import bacc
from concourse.bass_utils import run_bass_kernel_spmd

B = 8          # graphs == cores
N = 2048       # nodes per graph
F_IN = 256     # input features (= contraction dim K)
F_OUT = 256    # num_heads * d_head
P = 128        # SBUF/PSUM partitions
NTILE = 512    # PSUM bank free-dim (fp32)

KC = F_IN // P     # 2 contraction chunks
MC = F_OUT // P    # 2 output-feature chunks
NC_ = N // NTILE   # 4 node chunks
XSPLIT = 2         # node-dim halves per x DMA
XW = N // XSPLIT   # 1024

N_WARMUP_MM = 0    # disabled while bisecting device crash

# PE matmul dtype: float32 (exact, 4 cycles/row), float32r (reduced-precision
# single pass, 1 cycle/row at N=512, rel err ~1.4e-4), bfloat16 (1 cycle/row,
# half input DMA, rel err ~2.2e-3).
MATMUL_DTYPE = "float32r"

_module_cache = {}

# test.py reads this after calling kernel() to get profile/exec-time info.
LAST_RESULTS = None


def _build_module(mm_dtype: str) -> bass.Bass:
    if mm_dtype == "bfloat16":
        in_dt = mybir.dt.bfloat16
    elif mm_dtype == "float32r":
        in_dt = mybir.dt.float32r
    else:
        in_dt = mybir.dt.float32

    nc = bacc.Bacc(None, target_bir_lowering=False)
    xt = nc.dram_tensor("xt", [F_IN, N], in_dt, kind="ExternalInput")
    wc = nc.dram_tensor("wc", [F_IN, F_OUT], in_dt, kind="ExternalInput")
    yt = nc.dram_tensor("yt", [F_OUT, N], mybir.dt.float32, kind="ExternalOutput")

    # DRAM views with the k (contraction) chunk as an explicit axis so one
    # DMA can pack both k-chunks side by side in a single SBUF tile.
    # [256, C] -> [128p, 2k, C]
    wc_k = wc.rearrange("(a p) c -> p a c", p=P)
    xt_k = xt.rearrange("(a p) (j n) -> p j a n", p=P, j=XSPLIT)

    with tile.TileContext(nc) as tc:
        with (
            tc.tile_pool(name="wpool", bufs=1) as wpool,
            tc.tile_pool(name="xpool", bufs=1) as xpool,
            tc.tile_pool(name="ypool", bufs=1) as ypool,
            tc.tile_pool(name="warmpool", bufs=1) as warmpool,
            tc.tile_pool(name="pspool", bufs=1, space="PSUM") as pspool,
        ):
            # Scratch operands for PE warm-up (zeros; values are irrelevant).
            wu = warmpool.tile([P, NTILE], mybir.dt.float32, name="wu", tag="wu")
            nc.gpsimd.memset(wu[:], 0.0)
            wu_mm = wu[:].bitcast(in_dt)

            # w_sb[:, k*F_OUT + c] = Wc[k*128 + p, c]
            w_sb = wpool.tile([P, KC * F_OUT], in_dt, name="wsb", tag="wsb")
            nc.sync.dma_start(w_sb[:].rearrange("p (a c) -> p a c", a=KC), wc_k[:])

            # x_sb[j][:, k*XW + n] = X^T[k*128 + p, j*XW + n]
            x_sb = [
                xpool.tile([P, KC * XW], in_dt, name=f"x{j}", tag=f"x{j}")
                for j in range(XSPLIT)
            ]
            nc.sync.dma_start(
                x_sb[0][:].rearrange("p (a n) -> p a n", a=KC), xt_k[:, 0]
            )
            nc.scalar.dma_start(
                x_sb[1][:].rearrange("p (a n) -> p a n", a=KC), xt_k[:, 1]
            )

            ps = [
                [
                    pspool.tile(
                        [P, NTILE], mybir.dt.float32, name=f"ps{m}_{n}", tag=f"ps{m}_{n}"
                    )
                    for n in range(NC_)
                ]
                for m in range(MC)
            ]
            y_sb = [
                ypool.tile([P, N], mybir.dt.float32, name=f"y{m}", tag=f"y{m}")
                for m in range(MC)
            ]

            # PE clock warm-up on scratch data while the x DMAs are in
            # flight. Runs on ps[0][0] before its real accumulation group;
            # Tile's WAW tracking keeps program order.
            for _ in range(N_WARMUP_MM):
                nc.tensor.matmul(ps[0][0][:], wu_mm[:, :P], wu_mm, start=True, stop=True)

            # n-outer: both m-chunks of a node range finish together so their
            # output DMAs start as early as possible and overlap the x DMAs.
            for n in range(NC_):
                j = n // (NC_ // XSPLIT)
                noff = (n % (NC_ // XSPLIT)) * NTILE
                for m in range(MC):
                    for k in range(KC):
                        nc.tensor.matmul(
                            ps[m][n][:],
                            w_sb[:, k * F_OUT + m * P : k * F_OUT + m * P + P],
                            x_sb[j][:, k * XW + noff : k * XW + noff + NTILE],
                            start=(k == 0),
                            stop=(k == KC - 1),
                        )
                    # PSUM->SBUF eviction alternating DVE/ACT, then the
                    # [128,512] chunk flies out immediately on a queue that
                    # is not carrying input DMAs.
                    dst = y_sb[m][:, n * NTILE : (n + 1) * NTILE]
                    if (2 * n + m) % 2 == 0:
                        nc.vector.tensor_copy(dst, ps[m][n][:])
                    else:
                        nc.scalar.copy(dst, ps[m][n][:])
                    out_eng = nc.gpsimd if (2 * n + m) % 2 == 0 else nc.sync
                    out_eng.dma_start(
                        yt[m * P : (m + 1) * P, n * NTILE : (n + 1) * NTILE], dst
                    )
    nc.compile()
    return nc


def _get_module() -> bass.Bass:
    if MATMUL_DTYPE not in _module_cache:
        _module_cache[MATMUL_DTYPE] = _build_module(MATMUL_DTYPE)
    return _module_cache[MATMUL_DTYPE]


def kernel(h: np.ndarray, adj: np.ndarray, W: np.ndarray, **_unused) -> np.ndarray:
    global LAST_RESULTS
    h = np.asarray(h, dtype=np.float32)
    W = np.asarray(W, dtype=np.float32)
    # Wc[f, head*64+d] = W[head, f, d]
    wc = np.ascontiguousarray(W.transpose(1, 0, 2).reshape(F_IN, F_OUT))

    if MATMUL_DTYPE == "bfloat16":
        import ml_dtypes

        cast = lambda a: np.ascontiguousarray(a.astype(ml_dtypes.bfloat16))
    else:
        cast = np.ascontiguousarray

    wc_in = cast(wc)
    in_maps = [{"xt": cast(h[b].T), "wc": wc_in} for b in range(B)]
    nc = _get_module()
    res = run_bass_kernel_spmd(nc, in_maps, core_ids=list(range(B)))
    LAST_RESULTS = res

    out = np.empty((B, N, F_OUT), dtype=np.float32)
    for b in range(B):
        out[b] = res.results[b]["yt"].T
    return out
